# revision 54
# baseline (speedup 1.0000x reference)
"""Distributed AttentionHead kernel for 8 TRN2 NeuronCores.

Problem: qkv = x @ w.T ; q,k,v = split(qkv); scores[i,j] = k_i.q_j/sqrt(E),
mask keeps j >= i; out = softmax(scores) @ v.   B,N,H,E = 4,2048,1024,1024.

Sharding: core c = 2*b + s handles batch b; it owns the 8 row-tiles
{128*(2*lt+s) : lt in 0..7} (parity interleave => every core's attention
loop has j-extents (16,14,12,...,2) tiles => a single uniform SPMD graph).
Masks that differ between even/odd cores are passed as input *data*,
keeping the graph identical on all cores (required: collectives-free SPMD).

Algebraic restructure (host folds the weights, saving 2 of 5 GEMM stages):
  scores = k_i . q_j = x_i Wk^T Wq x_j^T = (X M) X^T,  M := Wk^T Wq  (host)
  out    = P (X Wv^T) = (P X) Wv^T = U Wv^T,           U := P X, own rows
so the device runs 4 GEMM groups: G^T = M^T X_own^T, scores = G X^T
(mask-skipped), U = P X, out = U Wv^T.  All operands arrive pre-cast to
bf16 from the host (halves HBM traffic); accumulation is fp32 in PSUM;
the output returns bf16 and is upcast on the host.

Per-core work: G 2.15 GF + scores ~2.4 + U ~2.4 + out 2.15 = ~9.1 GF
= ~278k PE cycles (116us at 2.4GHz); measured ~150us end-to-end with the
PE ~97% busy inside its span (rest: DMA-bound lead-in + drain barrier).

Measured dead ends (do not revisit without new evidence): fp8e4 DoubleRow
is 2.0x bf16 on HW (not the cost model's 4x) and plain-fp8 numerics blow
the 2e-2 budget (~3% per GEMM); dma_start_transpose costs ~4.8us per
[128,128] tile; big DMAs on sync/scalar queues stall the semaphore hub;
transposed-scores layout saves transposes but repays them in denominator
matmuls; supply-order tweaks beyond the current one are noise (+-1.5us
run-to-run variance).
"""
import os
import sys

sys.path.insert(0, "/opt/trn_rl_repo")

import numpy as np
import ml_dtypes

import concourse.mybir as mybir
from concourse import bacc
from concourse.tile import TileContext
from concourse.bass_utils import run_bass_kernel_spmd

B, N, H, E = 4, 2048, 1024, 1024
NT = N // 128          # 16 row tiles per batch
LT = 8                 # row tiles owned per core
BF = mybir.dt.bfloat16
F32 = mybir.dt.float32

_CACHE = {}
LAST_RESULT = None


def _build():
    nc = bacc.Bacc("TRN2", target_bir_lowering=False, debug=False, num_devices=8)

    # xn / wvT / wM arrive pre-packed on the host into partition-major
    # contiguous blocks (one long run per partition instead of 8-16 strided
    # 0.5-2KB runs), so their DMA descriptors stream at full rate.  xT stays
    # row-major: its score-chunk APs interleave own/other column tiles and
    # cannot be split across separate SBUF tiles without extra matmuls.
    xT_ext = nc.dram_tensor("xT", [H, N], BF, kind="ExternalInput")
    xn_ext = nc.dram_tensor("xnp", [128, NT * H], BF, kind="ExternalInput")
    wM0_ext = nc.dram_tensor("wM0", [128, 8 * 256], BF, kind="ExternalInput")
    wM1_ext = nc.dram_tensor("wM1", [128, 8 * 256], BF, kind="ExternalInput")
    wM2_ext = nc.dram_tensor("wM2", [128, 8 * 512], BF, kind="ExternalInput")
    wv_ext = nc.dram_tensor("wvp", [128, 8 * E], BF, kind="ExternalInput")
    am_ext = nc.dram_tensor("amask", [128, 768], F32, kind="ExternalInput")
    id_ext = nc.dram_tensor("ident", [128, 128], BF, kind="ExternalInput")
    out_ext = nc.dram_tensor("out", [LT, 128, 1024], BF, kind="ExternalOutput")

    xT_r = xT_ext.rearrange("(hs p) n -> p hs n", p=128)
    xn_r = xn_ext.rearrange("p (jt h) -> p jt h", jt=NT)
    wM0_r = wM0_ext.rearrange("p (hs c) -> p hs c", hs=8)
    wM1_r = wM1_ext.rearrange("p (hs c) -> p hs c", hs=8)
    wM2_r = wM2_ext.rearrange("p (hs c) -> p hs c", hs=8)
    wv_r = wv_ext.rearrange("p (hs o) -> p hs o", hs=8)

    with TileContext(nc) as tc:
        with (
            tc.tile_pool(name="consts", bufs=1) as consts,
            tc.tile_pool(name="bigx", bufs=1) as bigx,
            tc.tile_pool(name="qkv", bufs=1) as qkv,
            tc.tile_pool(name="pbuf", bufs=1) as pbuf,
            tc.tile_pool(name="pts", bufs=6) as ptsp,
            tc.tile_pool(name="ubuf", bufs=3) as ubuf,
            tc.tile_pool(name="utb", bufs=2) as utb,
            tc.tile_pool(name="outb", bufs=2) as outb,
            tc.tile_pool(name="smalls", bufs=3) as smalls,
            tc.tile_pool(name="acc", bufs=4, space="PSUM") as accp,
            tc.tile_pool(name="sc", bufs=2, space="PSUM") as scp,
            tc.tile_pool(name="tp", bufs=2, space="PSUM") as tpp,
        ):
            # Only the tiny ident const on the sync queue (issues ~2.5us in);
            # every big strided load MUST go through gpsimd — descriptor
            # generation on the sync/scalar sequencers runs inline and chokes
            # the semaphore hub (measured: a 1MB load on sync stalled 35us).
            ident = consts.tile([128, 128], BF)
            nc.sync.dma_start(out=ident, in_=id_ext[:, :])
            # amask is contiguous [128, 3KB] (128 descriptors) — small enough
            # for the sync queue, and it frees 384KB from the gpsimd supply
            # chain that gates the G stage.
            am_sb = consts.tile([128, 768], F32)
            nc.sync.dma_start(out=am_sb, in_=am_ext[:, :])

            # Warm-up matmuls on ident (only dependency: the first tiny DMA):
            # keeps the PE busy from ~8us until the first G inputs land
            # (~16us, DMA-bandwidth-bound) so the HAM clock gate reaches 8/8
            # and G starts at full 2.4 GHz with no idle gap.  128-free insts
            # quantize the warmup end finely (65ns) to minimize overshoot.
            wu_ps = accp.tile([128, 128], F32, tag="acc", name="wu_ps")
            for r in range(60):
                nc.tensor.matmul(wu_ps, ident, ident, start=True, stop=True)

            # Bulk loads, ordered so the first G psum group's inputs (wM
            # column chunk 0, xT own-column chunk 0) land first.
            wM_sb0 = bigx.tile([128, 8, 256], BF)
            nc.gpsimd.dma_start(out=wM_sb0, in_=wM0_r)
            xT_sb = bigx.tile([128, 8, N], BF)
            nc.gpsimd.dma_start(out=xT_sb[:, :, 0:512], in_=xT_r[:, :, 0:512])
            nc.gpsimd.dma_start(out=xT_sb[:, :, 512:1024], in_=xT_r[:, :, 512:1024])
            wM_sb1 = bigx.tile([128, 8, 256], BF)
            nc.gpsimd.dma_start(out=wM_sb1, in_=wM1_r)
            wM_sb2 = bigx.tile([128, 8, 512], BF)
            nc.gpsimd.dma_start(out=wM_sb2, in_=wM2_r)
            nc.gpsimd.dma_start(out=xT_sb[:, :, 1024:2048], in_=xT_r[:, :, 1024:2048])

            xn_sb = qkv.tile([128, NT, H], BF, tag="xn", name="xn_sb")
            nc.gpsimd.dma_start(out=xn_sb, in_=xn_r)
            xn = [xn_sb[:, t, :] for t in range(NT)]

            wvT_sb = bigx.tile([128, 8, E], BF)
            nc.gpsimd.dma_start(out=wvT_sb, in_=wv_r)

            TT = [qkv.tile([128, N // 2], BF, tag=f"TT{h}", name=f"TT{h}") for h in range(8)]

            # ---------------- G^T = M^T X_own^T  (own rows) ----------------
            # ht-pair outer, i0 inner: wM block b is first needed ~6.8us x b
            # into G, matching the DMA supply order (xT first, wM trickling
            # in) instead of demanding all wM columns in the first 14us.
            for hb in range(4):
              for i0 in range(0, N // 2, 512):
                for ht in (2 * hb, 2 * hb + 1):
                    if ht < 2:
                        lhsT = wM_sb0[:, :, 128 * ht:128 * ht + 128]
                    elif ht < 4:
                        lhsT = wM_sb1[:, :, 128 * (ht - 2):128 * (ht - 2) + 128]
                    else:
                        lhsT = wM_sb2[:, :, 128 * (ht - 4):128 * (ht - 4) + 128]
                    ps = accp.tile([128, 512], F32, tag="acc", name="ps_t")
                    for hs in range(8):
                        nc.tensor.matmul(
                            ps,
                            lhsT[:, hs, :],
                            xT_sb[:, hs, i0:i0 + 512],
                            start=hs == 0,
                            stop=hs == 7,
                        )
                    nc.vector.tensor_copy(out=TT[ht][:, i0:i0 + 512], in_=ps)

            # ---------------- attention ----------------
            # out = U Wv^T, normalized by the softmax denominator.  Emission
            # of row-tile li's out-proj is DEFERRED until after row-tile
            # li+1's score matmuls, so the PE fills li+1's exp->transpose
            # latency with li's out-proj instead of idling (matters most for
            # the last row-tile, whose chain would otherwise be exposed).
            def emit_out(li, ut, rden, split=False):
                # split=True (last row-tile only): normalize + DMA in 256-wide
                # pieces so the final bytes reach HBM sooner before the
                # end-of-kernel drain barrier.
                ob = outb.tile([128, 1024], BF, tag="ob", name=f"ob{li}")
                step = 256 if split else 512
                for eh in range(2):
                    po = accp.tile([128, 512], F32, tag="acc", name=f"po{li}_{eh}")
                    for hs in range(8):
                        nc.tensor.matmul(
                            po,
                            ut[:, hs, :],
                            wvT_sb[:, hs, 512 * eh:512 * eh + 512],
                            start=hs == 0,
                            stop=hs == 7,
                        )
                    for c0 in range(0, 512, step):
                        nc.scalar.mul(
                            ob[:, 512 * eh + c0:512 * eh + c0 + step],
                            po[:, c0:c0 + step],
                            rden,
                        )
                        nc.sync.dma_start(
                            out=out_ext[li, :, 512 * eh + c0:512 * eh + c0 + step],
                            in_=ob[:, 512 * eh + c0:512 * eh + c0 + step],
                        )

            pending = None
            for li in range(LT):
                ng = 8 - li           # j-extent in (own,other) tile pairs
                nj = NT - 2 * li      # 128-wide j tiles
                # score chunks: 512-wide where possible, one 256 tail if odd.
                # A 512 chunk at pair-base g covers p columns
                # [own g, own g+1, other g, other g+1]; a 256 chunk [own g,
                # other g].  jmap[u] = xn row-tile index of p's u-th 128-col.
                widths = [512] * (ng // 2) + [256] * (ng % 2)
                jmap = []
                for ci, w in enumerate(widths):
                    g = li + 2 * ci
                    if w == 512:
                        jmap += [g, g + 1, 8 + g, 8 + g + 1]
                    else:
                        jmap += [g, 8 + g]
                p = pbuf.tile([128, 128 * nj], BF, tag=f"p{li}", name=f"p{li}")
                asum = smalls.tile([128, 4], F32, tag="asum", name=f"asum{li}")
                xT_g = xT_sb[:, :, :].rearrange(
                    "p hs (two g c) -> p hs two g c", two=2, c=128
                )
                off = 0
                for ci, w in enumerate(widths):
                    g = li + 2 * ci
                    ps = scp.tile([128, w], F32, tag="sc", name=f"ps_s{li}_{ci}")
                    for hs in range(8):
                        rhs = xT_g[:, hs, :, g:g + w // 256, :]
                        nc.tensor.matmul(
                            ps,
                            TT[hs][:, 128 * li:128 * li + 128],
                            rhs,
                            start=hs == 0,
                            stop=hs == 7,
                        )
                    if ci == 0:
                        # additive mask: [0:512] holds the 512-chunk variant,
                        # [512:768] the 256-chunk variant (li=7 only).
                        mk = am_sb[:, 0:512] if w == 512 else am_sb[:, 512:768]
                        nc.vector.tensor_add(ps, ps, mk)
                    nc.scalar.activation(
                        out=p[:, off:off + w],
                        in_=ps,
                        func=mybir.ActivationFunctionType.Exp,
                        scale=float(1.0 / np.sqrt(E)),
                        accum_out=asum[:, ci:ci + 1],
                    )
                    off += w
                if pending is not None:
                    emit_out(*pending)
                    pending = None
                ut = utb.tile([128, 8, 128], BF, tag="ut", name=f"ut{li}")
                if li < 5:
                    # U = P X  (f32 accum in PSUM, bf16 out), then transpose U
                    pv0 = accp.tile([128, 512], F32, tag="acc", name=f"pv0_{li}")
                    pv1 = accp.tile([128, 512], F32, tag="acc", name=f"pv1_{li}")
                    for u in range(nj):
                        tp = tpp.tile([128, 128], BF, tag="tp", name=f"tp{li}_{u}")
                        nc.tensor.transpose(tp, p[:, 128 * u:128 * u + 128], ident)
                        pt = ptsp.tile([128, 128], BF, tag="pts", name=f"pt{li}_{u}")
                        nc.vector.tensor_copy(out=pt, in_=tp)
                        jt = jmap[u]
                        nc.tensor.matmul(
                            pv0, pt, xn[jt][:, 0:512], start=u == 0, stop=u == nj - 1
                        )
                        nc.tensor.matmul(
                            pv1, pt, xn[jt][:, 512:1024], start=u == 0, stop=u == nj - 1
                        )
                    usb = ubuf.tile([128, H], BF, tag="u", name=f"u{li}")
                    nc.scalar.copy(out=usb[:, 0:512], in_=pv0)
                    nc.scalar.copy(out=usb[:, 512:1024], in_=pv1)
                    for hs in range(8):
                        tp = tpp.tile([128, 128], BF, tag="tp", name=f"tpu{li}_{hs}")
                        nc.tensor.transpose(tp, usb[:, 128 * hs:128 * hs + 128], ident)
                        nc.vector.tensor_copy(out=ut[:, hs, :], in_=tp)
                else:
                    # small j-window: accumulate U^T directly (shorter serial
                    # chain; PE has slack here)
                    pts_list = []
                    for u in range(nj):
                        tp = tpp.tile([128, 128], BF, tag="tp", name=f"tp{li}_{u}")
                        nc.tensor.transpose(tp, p[:, 128 * u:128 * u + 128], ident)
                        pt = ptsp.tile([128, 128], BF, tag="pts", name=f"pt{li}_{u}")
                        nc.vector.tensor_copy(out=pt, in_=tp)
                        pts_list.append(pt)
                    for ht in range(8):
                        up = accp.tile([128, 128], F32, tag="acc", name=f"up{li}_{ht}")
                        for u in range(nj):
                            nc.tensor.matmul(
                                up,
                                xn[jmap[u]][:, 128 * ht:128 * ht + 128],
                                pts_list[u],
                                start=u == 0,
                                stop=u == nj - 1,
                            )
                        nc.vector.tensor_copy(out=ut[:, ht, :], in_=up)
                den = smalls.tile([128, 1], F32, tag="den", name=f"den{li}")
                nc.vector.reduce_sum(den, asum[:, 0:len(widths)], axis=mybir.AxisListType.X)
                rden = smalls.tile([128, 1], F32, tag="rden", name=f"rden{li}")
                nc.vector.reciprocal(rden, den)
                pending = (li, ut, rden)
            emit_out(*pending, split=True)

    nc.compile()
    return nc


def _amask(s: int) -> np.ndarray:
    # Additive masks for the first score chunk of each row-tile li.
    # Columns 0:512 = 512-wide chunk [own li | own li+1 | other li |
    # other li+1]; columns 512:768 = 256-wide chunk [own li | other li]
    # (li=7).  Own diagonal tile gets the triangular mask; the partner
    # tile of slot li is global tile 2li+(1-s): above the diagonal for
    # s=0 (keep), below for s=1 (mask out).  All other tiles sit above
    # the diagonal -> keep.
    m = np.zeros((128, 768), dtype=np.float32)
    i = np.arange(128)[:, None]
    j = np.arange(128)[None, :]
    tri = np.where(j >= i, 0.0, -1e9).astype(np.float32)
    m[:, 0:128] = tri
    if s == 1:
        m[:, 256:384] = -1e9
    m[:, 512:640] = tri
    if s == 1:
        m[:, 640:768] = -1e9
    return m


def _perm(s: int) -> np.ndarray:
    own = [2 * u + s for u in range(8)]
    other = [2 * u + 1 - s for u in range(8)]
    return np.array(own + other)


def kernel(input: np.ndarray, w: np.ndarray) -> np.ndarray:
    global LAST_RESULT
    if "nc" not in _CACHE:
        _CACHE["nc"] = _build()
    nc = _CACHE["nc"]

    input = np.ascontiguousarray(input, dtype=np.float32)
    w = np.ascontiguousarray(w, dtype=np.float32)
    BF16 = ml_dtypes.bfloat16

    def pack(a, nslab):
        # [nslab*128, F] row-major -> [128, nslab*F] partition-major so the
        # device DMA reads one contiguous run per partition.
        ns, f = a.shape
        return np.ascontiguousarray(
            a.reshape(nslab, 128, f).transpose(1, 0, 2).reshape(128, nslab * f)
        )

    # Host-side weight folds (fp32 matmul, then bf16 cast for the device).
    wM = (w[E:2 * E, :].T @ w[0:E, :]).astype(BF16)     # [H, H] = Wk^T Wq
    wM3 = wM.reshape(8, 128, H).transpose(1, 0, 2)      # [128, 8, H]
    wM0 = np.ascontiguousarray(wM3[:, :, 0:256].reshape(128, 8 * 256))
    wM1 = np.ascontiguousarray(wM3[:, :, 256:512].reshape(128, 8 * 256))
    wM2 = np.ascontiguousarray(wM3[:, :, 512:1024].reshape(128, 8 * 512))
    wvp = pack(w[2 * E:3 * E, :].T.astype(BF16), 8)     # [128, 8*E]
    ident = np.eye(128, dtype=BF16)

    in_maps = []
    for c in range(8):
        b, s = divmod(c, 2)
        perm = _perm(s)
        xt3 = input[b].T.reshape(H, NT, 128)            # [H, 16, 128]
        xT = np.ascontiguousarray(
            xt3[:, perm, :].reshape(H, N).astype(BF16)
        )                                               # [H, N] col-tiles permuted
        xn3 = input[b].reshape(NT, 128, H)
        xnp = pack(
            xn3[perm].reshape(N, H).astype(BF16), NT
        )                                               # [128, NT*H] packed
        in_maps.append(
            {
                "xT": xT,
                "xnp": xnp,
                "wM0": wM0,
                "wM1": wM1,
                "wM2": wM2,
                "wvp": wvp,
                "amask": _amask(s),
                "ident": ident,
            }
        )

    trace = bool(int(os.environ.get("KERNEL_TRACE", "0")))
    res = run_bass_kernel_spmd(nc, in_maps, core_ids=list(range(8)), trace=trace)
    LAST_RESULT = res

    out = np.empty((B, N, E), dtype=np.float32)
    for c in range(8):
        b, s = divmod(c, 2)
        o = res.results[c]["out"]                       # [LT, 128, 1024] bf16
        for lt in range(LT):
            r0 = 128 * (2 * lt + s)
            out[b, r0:r0 + 128, :] = o[lt].astype(np.float32)
    return out


# revision 55
# speedup vs baseline: 1.1847x; 1.1847x over previous
"""Distributed AttentionHead kernel for 8 TRN2 NeuronCores.

Problem: qkv = x @ w.T ; q,k,v = split(qkv); scores[i,j] = k_i.q_j/sqrt(E),
mask keeps j >= i; out = softmax(scores) @ v.   B,N,H,E = 4,2048,1024,1024.

Sharding: core c = 2*b + s handles batch b; it owns the 8 row-tiles
{128*(2*lt+s) : lt in 0..7} (parity interleave => every core's attention
loop has j-extents (16,14,12,...,2) tiles => a single uniform SPMD graph).
Masks that differ between even/odd cores are passed as input *data*,
keeping the graph identical on all cores (required: collectives-free SPMD).

Algebraic restructure (host folds the weights, saving 2 of 5 GEMM stages):
  scores = k_i . q_j = x_i Wk^T Wq x_j^T = (X M) X^T,  M := Wk^T Wq  (host)
  out    = P (X Wv^T) = (P X) Wv^T = U Wv^T,           U := P X, own rows
so the device runs 4 GEMM groups: G^T = M^T X_own^T, scores = G X^T
(mask-skipped), U = P X, out = U Wv^T.  All operands arrive pre-cast to
bf16 from the host (halves HBM traffic); accumulation is fp32 in PSUM;
the output returns bf16 and is upcast on the host.

Per-core work: G 2.15 GF + scores ~2.4 + U ~2.4 + out 2.15 = ~9.1 GF
= ~278k PE cycles (116us at 2.4GHz); measured ~150us end-to-end with the
PE ~97% busy inside its span (rest: DMA-bound lead-in + drain barrier).

Measured dead ends (do not revisit without new evidence): fp8e4 DoubleRow
is 2.0x bf16 on HW (not the cost model's 4x) and plain-fp8 numerics blow
the 2e-2 budget (~3% per GEMM); dma_start_transpose costs ~4.8us per
[128,128] tile; big DMAs on sync/scalar queues stall the semaphore hub;
transposed-scores layout saves transposes but repays them in denominator
matmuls; supply-order tweaks beyond the current one are noise (+-1.5us
run-to-run variance).
"""
import os
import sys

sys.path.insert(0, "/opt/trn_rl_repo")

import numpy as np
import ml_dtypes

import concourse.mybir as mybir
from concourse import bacc
from concourse.tile import TileContext
from concourse.bass_utils import run_bass_kernel_spmd

B, N, H, E = 4, 2048, 1024, 1024
NT = N // 128          # 16 row tiles per batch
LT = 8                 # row tiles owned per core
BF = mybir.dt.bfloat16
F32 = mybir.dt.float32

_CACHE = {}
LAST_RESULT = None


def _build():
    nc = bacc.Bacc("TRN2", target_bir_lowering=False, debug=False, num_devices=8)

    # xn / wvT / wM arrive pre-packed on the host into partition-major
    # contiguous blocks (one long run per partition instead of 8-16 strided
    # 0.5-2KB runs), so their DMA descriptors stream at full rate.  xT stays
    # row-major: its score-chunk APs interleave own/other column tiles and
    # cannot be split across separate SBUF tiles without extra matmuls.
    xT_ext = nc.dram_tensor("xT", [H, N], BF, kind="ExternalInput")
    xn_ext = nc.dram_tensor("xnp", [128, NT * H], BF, kind="ExternalInput")
    wM0_ext = nc.dram_tensor("wM0", [128, 8 * 256], BF, kind="ExternalInput")
    wM1_ext = nc.dram_tensor("wM1", [128, 8 * 768], BF, kind="ExternalInput")
    wv_ext = nc.dram_tensor("wvp", [128, 8 * E], BF, kind="ExternalInput")
    am_ext = nc.dram_tensor("amask", [128, 768], F32, kind="ExternalInput")
    id_ext = nc.dram_tensor("ident", [128, 128], BF, kind="ExternalInput")
    out_ext = nc.dram_tensor("out", [LT, 128, 1024], BF, kind="ExternalOutput")

    xT_r = xT_ext.rearrange("(hs p) n -> p hs n", p=128)
    xn_r = xn_ext.rearrange("p (jt h) -> p jt h", jt=NT)
    wM0_r = wM0_ext.rearrange("p (hs c) -> p hs c", hs=8)
    wM1_r = wM1_ext.rearrange("p (hs c) -> p hs c", hs=8)
    wv_r = wv_ext.rearrange("p (hs o) -> p hs o", hs=8)

    with TileContext(nc) as tc:
        with (
            tc.tile_pool(name="consts", bufs=1) as consts,
            tc.tile_pool(name="bigx", bufs=1) as bigx,
            tc.tile_pool(name="qkv", bufs=1) as qkv,
            tc.tile_pool(name="pbuf", bufs=1) as pbuf,
            tc.tile_pool(name="pts", bufs=6) as ptsp,
            tc.tile_pool(name="ubuf", bufs=3) as ubuf,
            tc.tile_pool(name="utb", bufs=2) as utb,
            tc.tile_pool(name="outb", bufs=2) as outb,
            tc.tile_pool(name="smalls", bufs=3) as smalls,
            tc.tile_pool(name="acc", bufs=4, space="PSUM") as accp,
            tc.tile_pool(name="sc", bufs=2, space="PSUM") as scp,
            tc.tile_pool(name="tp", bufs=2, space="PSUM") as tpp,
        ):
            # Only the tiny ident const on the sync queue (issues ~2.5us in);
            # every big strided load MUST go through gpsimd — descriptor
            # generation on the sync/scalar sequencers runs inline and chokes
            # the semaphore hub (measured: a 1MB load on sync stalled 35us).
            ident = consts.tile([128, 128], BF)
            nc.sync.dma_start(out=ident, in_=id_ext[:, :])
            # amask is contiguous [128, 3KB] (128 descriptors) — small enough
            # for the sync queue, and it frees 384KB from the gpsimd supply
            # chain that gates the G stage.
            am_sb = consts.tile([128, 768], F32)
            nc.sync.dma_start(out=am_sb, in_=am_ext[:, :])

            # Warm-up matmuls on ident (only dependency: the first tiny DMA):
            # keeps the PE busy from ~8us until the first G inputs land
            # (~16us, DMA-bandwidth-bound) so the HAM clock gate reaches 8/8
            # and G starts at full 2.4 GHz with no idle gap.  128-free insts
            # quantize the warmup end finely (65ns) to minimize overshoot.
            wu_ps = accp.tile([128, 128], F32, tag="acc", name="wu_ps")
            for r in range(60):
                nc.tensor.matmul(wu_ps, ident, ident, start=True, stop=True)

            # Bulk loads, ordered so the first G psum group's inputs (wM
            # column chunk 0, xT own-column chunk 0) land first.
            wM_sb0 = bigx.tile([128, 8, 256], BF)
            nc.gpsimd.dma_start(out=wM_sb0, in_=wM0_r)
            xT_sb = bigx.tile([128, 8, N], BF)
            nc.gpsimd.dma_start(out=xT_sb[:, :, 0:512], in_=xT_r[:, :, 0:512])
            wM_sb1 = bigx.tile([128, 8, 768], BF)
            nc.gpsimd.dma_start(out=wM_sb1, in_=wM1_r)
            nc.gpsimd.dma_start(out=xT_sb[:, :, 512:1024], in_=xT_r[:, :, 512:1024])
            nc.gpsimd.dma_start(out=xT_sb[:, :, 1024:2048], in_=xT_r[:, :, 1024:2048])

            xn_sb = qkv.tile([128, NT, H], BF, tag="xn", name="xn_sb")
            nc.gpsimd.dma_start(out=xn_sb, in_=xn_r)
            xn = [xn_sb[:, t, :] for t in range(NT)]

            wvT_sb = bigx.tile([128, 8, E], BF)
            nc.gpsimd.dma_start(out=wvT_sb, in_=wv_r)

            TT = [qkv.tile([128, N // 2], BF, tag=f"TT{h}", name=f"TT{h}") for h in range(8)]

            # ---------------- G^T = M^T X_own^T  (own rows) ----------------
            for i0 in range(0, N // 2, 512):
                for ht in range(8):
                    if ht < 2:
                        lhsT = wM_sb0[:, :, 128 * ht:128 * ht + 128]
                    else:
                        lhsT = wM_sb1[:, :, 128 * (ht - 2):128 * (ht - 2) + 128]
                    ps = accp.tile([128, 512], F32, tag="acc", name="ps_t")
                    for hs in range(8):
                        nc.tensor.matmul(
                            ps,
                            lhsT[:, hs, :],
                            xT_sb[:, hs, i0:i0 + 512],
                            start=hs == 0,
                            stop=hs == 7,
                        )
                    nc.vector.tensor_copy(out=TT[ht][:, i0:i0 + 512], in_=ps)

            # ---------------- attention ----------------
            # out = U Wv^T, normalized by the softmax denominator.  Emission
            # of row-tile li's out-proj is DEFERRED until after row-tile
            # li+1's score matmuls, so the PE fills li+1's exp->transpose
            # latency with li's out-proj instead of idling (matters most for
            # the last row-tile, whose chain would otherwise be exposed).
            def emit_out(li, ut, rden, split=False):
                # split=True (last row-tile only): normalize + DMA in 256-wide
                # pieces so the final bytes reach HBM sooner before the
                # end-of-kernel drain barrier.
                ob = outb.tile([128, 1024], BF, tag="ob", name=f"ob{li}")
                step = 256 if split else 512
                for eh in range(2):
                    po = accp.tile([128, 512], F32, tag="acc", name=f"po{li}_{eh}")
                    for hs in range(8):
                        nc.tensor.matmul(
                            po,
                            ut[:, hs, :],
                            wvT_sb[:, hs, 512 * eh:512 * eh + 512],
                            start=hs == 0,
                            stop=hs == 7,
                        )
                    for c0 in range(0, 512, step):
                        nc.scalar.mul(
                            ob[:, 512 * eh + c0:512 * eh + c0 + step],
                            po[:, c0:c0 + step],
                            rden,
                        )
                        nc.sync.dma_start(
                            out=out_ext[li, :, 512 * eh + c0:512 * eh + c0 + step],
                            in_=ob[:, 512 * eh + c0:512 * eh + c0 + step],
                        )

            pending = None
            for li in range(LT):
                ng = 8 - li           # j-extent in (own,other) tile pairs
                nj = NT - 2 * li      # 128-wide j tiles
                # score chunks: 512-wide where possible, one 256 tail if odd.
                # A 512 chunk at pair-base g covers p columns
                # [own g, own g+1, other g, other g+1]; a 256 chunk [own g,
                # other g].  jmap[u] = xn row-tile index of p's u-th 128-col.
                widths = [512] * (ng // 2) + [256] * (ng % 2)
                jmap = []
                for ci, w in enumerate(widths):
                    g = li + 2 * ci
                    if w == 512:
                        jmap += [g, g + 1, 8 + g, 8 + g + 1]
                    else:
                        jmap += [g, 8 + g]
                p = pbuf.tile([128, 128 * nj], BF, tag=f"p{li}", name=f"p{li}")
                asum = smalls.tile([128, 4], F32, tag="asum", name=f"asum{li}")
                xT_g = xT_sb[:, :, :].rearrange(
                    "p hs (two g c) -> p hs two g c", two=2, c=128
                )
                off = 0
                for ci, w in enumerate(widths):
                    g = li + 2 * ci
                    ps = scp.tile([128, w], F32, tag="sc", name=f"ps_s{li}_{ci}")
                    for hs in range(8):
                        rhs = xT_g[:, hs, :, g:g + w // 256, :]
                        nc.tensor.matmul(
                            ps,
                            TT[hs][:, 128 * li:128 * li + 128],
                            rhs,
                            start=hs == 0,
                            stop=hs == 7,
                        )
                    if ci == 0:
                        # additive mask: [0:512] holds the 512-chunk variant,
                        # [512:768] the 256-chunk variant (li=7 only).
                        mk = am_sb[:, 0:512] if w == 512 else am_sb[:, 512:768]
                        nc.vector.tensor_add(ps, ps, mk)
                    nc.scalar.activation(
                        out=p[:, off:off + w],
                        in_=ps,
                        func=mybir.ActivationFunctionType.Exp,
                        scale=float(1.0 / np.sqrt(E)),
                        accum_out=asum[:, ci:ci + 1],
                    )
                    off += w
                if pending is not None:
                    emit_out(*pending)
                    pending = None
                ut = utb.tile([128, 8, 128], BF, tag="ut", name=f"ut{li}")
                if li < 5:
                    # U = P X  (f32 accum in PSUM, bf16 out), then transpose U
                    pv0 = accp.tile([128, 512], F32, tag="acc", name=f"pv0_{li}")
                    pv1 = accp.tile([128, 512], F32, tag="acc", name=f"pv1_{li}")
                    for u in range(nj):
                        tp = tpp.tile([128, 128], BF, tag="tp", name=f"tp{li}_{u}")
                        nc.tensor.transpose(tp, p[:, 128 * u:128 * u + 128], ident)
                        pt = ptsp.tile([128, 128], BF, tag="pts", name=f"pt{li}_{u}")
                        nc.vector.tensor_copy(out=pt, in_=tp)
                        jt = jmap[u]
                        nc.tensor.matmul(
                            pv0, pt, xn[jt][:, 0:512], start=u == 0, stop=u == nj - 1
                        )
                        nc.tensor.matmul(
                            pv1, pt, xn[jt][:, 512:1024], start=u == 0, stop=u == nj - 1
                        )
                    usb = ubuf.tile([128, H], BF, tag="u", name=f"u{li}")
                    nc.scalar.copy(out=usb[:, 0:512], in_=pv0)
                    nc.scalar.copy(out=usb[:, 512:1024], in_=pv1)
                    for hs in range(8):
                        tp = tpp.tile([128, 128], BF, tag="tp", name=f"tpu{li}_{hs}")
                        nc.tensor.transpose(tp, usb[:, 128 * hs:128 * hs + 128], ident)
                        nc.vector.tensor_copy(out=ut[:, hs, :], in_=tp)
                else:
                    # small j-window: accumulate U^T directly (shorter serial
                    # chain; PE has slack here)
                    pts_list = []
                    for u in range(nj):
                        tp = tpp.tile([128, 128], BF, tag="tp", name=f"tp{li}_{u}")
                        nc.tensor.transpose(tp, p[:, 128 * u:128 * u + 128], ident)
                        pt = ptsp.tile([128, 128], BF, tag="pts", name=f"pt{li}_{u}")
                        nc.vector.tensor_copy(out=pt, in_=tp)
                        pts_list.append(pt)
                    for ht in range(8):
                        up = accp.tile([128, 128], F32, tag="acc", name=f"up{li}_{ht}")
                        for u in range(nj):
                            nc.tensor.matmul(
                                up,
                                xn[jmap[u]][:, 128 * ht:128 * ht + 128],
                                pts_list[u],
                                start=u == 0,
                                stop=u == nj - 1,
                            )
                        nc.vector.tensor_copy(out=ut[:, ht, :], in_=up)
                den = smalls.tile([128, 1], F32, tag="den", name=f"den{li}")
                nc.vector.reduce_sum(den, asum[:, 0:len(widths)], axis=mybir.AxisListType.X)
                rden = smalls.tile([128, 1], F32, tag="rden", name=f"rden{li}")
                nc.vector.reciprocal(rden, den)
                pending = (li, ut, rden)
            emit_out(*pending, split=True)

    nc.compile()
    return nc


def _amask(s: int) -> np.ndarray:
    # Additive masks for the first score chunk of each row-tile li.
    # Columns 0:512 = 512-wide chunk [own li | own li+1 | other li |
    # other li+1]; columns 512:768 = 256-wide chunk [own li | other li]
    # (li=7).  Own diagonal tile gets the triangular mask; the partner
    # tile of slot li is global tile 2li+(1-s): above the diagonal for
    # s=0 (keep), below for s=1 (mask out).  All other tiles sit above
    # the diagonal -> keep.
    m = np.zeros((128, 768), dtype=np.float32)
    i = np.arange(128)[:, None]
    j = np.arange(128)[None, :]
    tri = np.where(j >= i, 0.0, -1e9).astype(np.float32)
    m[:, 0:128] = tri
    if s == 1:
        m[:, 256:384] = -1e9
    m[:, 512:640] = tri
    if s == 1:
        m[:, 640:768] = -1e9
    return m


def _perm(s: int) -> np.ndarray:
    own = [2 * u + s for u in range(8)]
    other = [2 * u + 1 - s for u in range(8)]
    return np.array(own + other)


def kernel(input: np.ndarray, w: np.ndarray) -> np.ndarray:
    global LAST_RESULT
    if "nc" not in _CACHE:
        _CACHE["nc"] = _build()
    nc = _CACHE["nc"]

    input = np.ascontiguousarray(input, dtype=np.float32)
    w = np.ascontiguousarray(w, dtype=np.float32)
    BF16 = ml_dtypes.bfloat16

    def pack(a, nslab):
        # [nslab*128, F] row-major -> [128, nslab*F] partition-major so the
        # device DMA reads one contiguous run per partition.
        ns, f = a.shape
        return np.ascontiguousarray(
            a.reshape(nslab, 128, f).transpose(1, 0, 2).reshape(128, nslab * f)
        )

    # Host-side weight folds (fp32 matmul, then bf16 cast for the device).
    wM = (w[E:2 * E, :].T @ w[0:E, :]).astype(BF16)     # [H, H] = Wk^T Wq
    wM3 = wM.reshape(8, 128, H).transpose(1, 0, 2)      # [128, 8, H]
    wM0 = np.ascontiguousarray(wM3[:, :, 0:256].reshape(128, 8 * 256))
    wM1 = np.ascontiguousarray(wM3[:, :, 256:1024].reshape(128, 8 * 768))
    wvp = pack(w[2 * E:3 * E, :].T.astype(BF16), 8)     # [128, 8*E]
    ident = np.eye(128, dtype=BF16)

    in_maps = []
    for c in range(8):
        b, s = divmod(c, 2)
        perm = _perm(s)
        xt3 = input[b].T.reshape(H, NT, 128)            # [H, 16, 128]
        xT = np.ascontiguousarray(
            xt3[:, perm, :].reshape(H, N).astype(BF16)
        )                                               # [H, N] col-tiles permuted
        xn3 = input[b].reshape(NT, 128, H)
        xnp = pack(
            xn3[perm].reshape(N, H).astype(BF16), NT
        )                                               # [128, NT*H] packed
        in_maps.append(
            {
                "xT": xT,
                "xnp": xnp,
                "wM0": wM0,
                "wM1": wM1,
                "wvp": wvp,
                "amask": _amask(s),
                "ident": ident,
            }
        )

    trace = bool(int(os.environ.get("KERNEL_TRACE", "0")))
    res = run_bass_kernel_spmd(nc, in_maps, core_ids=list(range(8)), trace=trace)
    LAST_RESULT = res

    out = np.empty((B, N, E), dtype=np.float32)
    for c in range(8):
        b, s = divmod(c, 2)
        o = res.results[c]["out"]                       # [LT, 128, 1024] bf16
        for lt in range(LT):
            r0 = 128 * (2 * lt + s)
            out[b, r0:r0 + 128, :] = o[lt].astype(np.float32)
    return out
